# revision 28
# baseline (speedup 1.0000x reference)
"""Trainium2 Bass kernel for nn_AttentionBlock (RMSNorm + QKV + causal
attention with softmax over the QUERY axis + output projection).

Sharding: data-parallel over batch. B=8 -> one batch element per NeuronCore,
no collectives.  Measured ~234 us HW exec (vs 415 us f32r baseline),
rel err 8.7e-3 vs the 2e-2 gate.

Design notes:
  * All matmul operands are bf16.  On this PE, bf16 and f32r both stream
    ~1 col/cycle (N=512 MM->MM issue gap measured 257 ns), so the bf16 win
    is NOT matmul throughput: it halves DMA/SBUF/DVE traffic, halves
    LDWEIGHTS (FWL), and -- crucially -- drops PE power enough that the
    HAM duty-cycle throttle (K=4/8 oscillation that cost the f32r baseline
    ~100 us) never engages.  fp32 is kept for all per-row scalars (1/rms,
    1/rowsum): rounding those to bf16 is a per-row systematic error that
    exp() amplifies (measured 1.5e-2 vs 8.7e-3 end-to-end).
  * Softmax work is lagged one head-pair behind the PE: during pair t the
    PE computes scores(t), the attn@V matmuls of pair t-1 (whose exp/vsc
    finished last iteration), and the QK projection of pair t+1, plus
    leftover V-chunk work -- so the PE never waits on the Scalar engine's
    exp chain.
  * Scores live transposed (sT[k, q], k on partitions) so the per-key
    softmax-over-q is a free-axis row sum: ACT exp reads the [128, width]
    PSUM row ([P, S] 2-bank tiles) and accum_out yields the row sums.
    For k-tiles >= 4 (width <= 512) both heads of the pair share one
    2-bank tile and a single wide exp covers them ([P, 2, width] strided
    AP); their row sums come from one DVE tensor_reduce.  Normalization
    is folded into V (vsc = V * 1/rowsum), never touching the attn tiles.
  * Causal mask: DVE adds a precomputed -1e30 upper-triangle onto the
    diagonal 128x128 block in PSUM (one 2-head strided add for merged
    tiles).  NOTE: a PE-side mask matmul (triu as weights, identity
    moving) also works but wastes PE issue slots; N=128 matmuls cost
    ~129 ns regardless (LDWEIGHTS-bound).
  * RMSNorm transposes write 4 x [128,128] per PSUM bank so one DVE copy
    drains four; QK projection q-chunk 0 is emitted right after the first
    transpose half so the PE isn't idle while tiles 4-7 normalize.
  * Output is written bf16 and upcast on the host (harness compares fp32).

Pitfalls hit on real HW (keep in mind when editing):
  * nc.vector.tensor_tensor_reduce crashed the device (NRT INTERNAL) --
    avoid; use ACT Square+accum_out for sum-of-squares.
  * Issuing the x DMAs from the GpSimd DGE queue added ~6 us of queue
    drain to the epilogue for no startup gain.
  * Moving the output projection's first 6 pair-contractions into the
    attention iterations (PSUM->SBUF partials + final DVE add) was a net
    LOSS: the last iterations have little true PE idle, and the tail adds
    serialize on DVE.
"""

import numpy as np
import ml_dtypes
from contextlib import ExitStack

import concourse.bacc as bacc
import concourse.bass as bass
import concourse.tile as tile
from concourse import mybir
from concourse.bass_utils import run_bass_kernel_spmd

B, S, DM, H, DH = 8, 1024, 1024, 16, 64
P = 128
EPS = 1.1920929e-07
NEG = -1e30
F32 = mybir.dt.float32
BF = mybir.dt.bfloat16
NS = S // P      # 8 s-tiles (also k-tiles)
ND = DM // P     # 8 d-chunks
NPAIR = H // 2   # 8 head pairs
QCH = 512        # moving-dim chunk (one PSUM bank of fp32)
NQC = S // QCH   # 2 q chunks

# merge both heads of a pair into one wide exp for k-tiles >= 4
MERGE_HI = True

BF_NP = ml_dtypes.bfloat16


def build_program():
    nc = bacc.Bacc("TRN2", target_bir_lowering=False, debug=False)

    xb = nc.dram_tensor("xb", [S, DM], BF, kind="ExternalInput").ap()
    # (ft, dd, dk, f): ft 0-7 = Q pair tiles, 8-15 = K pair tiles
    wqk = nc.dram_tensor("wqk", [2 * NPAIR, P, ND, P], BF, kind="ExternalInput").ap()
    wv = nc.dram_tensor("wv", [ND, P, DM], BF, kind="ExternalInput").ap()
    wo = nc.dram_tensor("wo", [ND, P, DM], BF, kind="ExternalInput").ap()
    ident = nc.dram_tensor("ident", [P, P], BF, kind="ExternalInput").ap()
    triu_neg = nc.dram_tensor("triu_neg", [P, P], F32, kind="ExternalInput").ap()
    out = nc.dram_tensor("out", [S, DM], BF, kind="ExternalOutput").ap()

    with tile.TileContext(nc) as tc:
        with ExitStack() as ctx:
            _build_body(ctx, tc, xb, wqk, wv, wo, ident, triu_neg, out)
    nc.compile()
    return nc


def _build_body(ctx, tc, xb, wqk, wv, wo, ident, triu_neg, out):
    nc = tc.nc
    AF = mybir.ActivationFunctionType
    ALU = mybir.AluOpType

    singles = ctx.enter_context(tc.tile_pool(name="singles", bufs=1))
    xv = ctx.enter_context(tc.tile_pool(name="xv", bufs=1))      # x then V
    sqp = ctx.enter_context(tc.tile_pool(name="sqp", bufs=2))    # square scratch
    nt = ctx.enter_context(tc.tile_pool(name="nt", bufs=1))      # normT
    w8 = ctx.enter_context(tc.tile_pool(name="w8", bufs=1))      # wv then wo
    wqks = ctx.enter_context(tc.tile_pool(name="wqks", bufs=4))  # wqk stream
    qkp = ctx.enter_context(tc.tile_pool(name="qkp", bufs=3))    # qt/kt tiles
    attnp = ctx.enter_context(tc.tile_pool(name="attnp", bufs=4))
    vscp = ctx.enter_context(tc.tile_pool(name="vscp", bufs=2))
    ztp = ctx.enter_context(tc.tile_pool(name="ztp", bufs=1))
    scr = ctx.enter_context(tc.tile_pool(name="scr", bufs=2))    # out tiles
    sm = ctx.enter_context(tc.tile_pool(name="sm", bufs=4))      # small stats
    ps = ctx.enter_context(tc.tile_pool(name="ps", bufs=1, space="PSUM"))

    # ---------------- Phase A: DMA + RMSNorm + transpose ----------------
    x_ts = []
    for st in range(NS):
        x_t = xv.tile([P, DM], BF, tag=f"b{st}", name=f"x{st}")
        nc.sync.dma_start(out=x_t, in_=xb[st * P:(st + 1) * P, :])
        x_ts.append(x_t)
    ident_sb = singles.tile([P, P], BF, tag="ident")
    nc.sync.dma_start(out=ident_sb, in_=ident)
    triu_sb = singles.tile([P, P], F32, tag="triu")
    nc.sync.dma_start(out=triu_sb, in_=triu_neg)
    eps_sb = singles.tile([P, 1], F32, tag="eps")
    nc.vector.memset(eps_sb, EPS)
    # preload all three ACT function tables while the x DMA is in flight so
    # no table load lands in the norm/exp critical chain
    warm = sm.tile([P, 1], F32, tag="warm", bufs=2, name="warm")
    nc.scalar.activation(out=warm, in_=eps_sb, func=AF.Square)
    nc.scalar.activation(out=warm, in_=eps_sb, func=AF.Sqrt)
    nc.scalar.activation(out=warm, in_=eps_sb, func=AF.Exp)
    # weight prefetch behind x on the DMA queue: QK pairs 0/1, then Wv
    wqk_pre = {}

    def wqk_dma(ft):
        w_t = wqks.tile([P, ND, P], BF, tag="wqk", name=f"wqk{ft}")
        nc.sync.dma_start(out=w_t, in_=wqk[ft])
        wqk_pre[ft] = w_t

    for ft in (0, NPAIR, 1, NPAIR + 1):
        wqk_dma(ft)
    wv_sb = []
    for dk in range(ND):
        w_t = w8.tile([P, DM], BF, tag=f"w{dk}", name=f"wv{dk}")
        nc.sync.dma_start(out=w_t, in_=wv[dk])
        wv_sb.append(w_t)

    ntile = nt.tile([P, ND, S], BF, tag="nt", name="normT")
    normT = [ntile[:, dk, :] for dk in range(ND)]

    def norm_tile(st):
        x_t = x_ts[st]
        ssum = sm.tile([P, 1], F32, tag="ssum", bufs=2, name=f"ssum{st}")
        sq = sqp.tile([P, DM], BF, tag="sq", name=f"sq{st}")
        nc.scalar.activation(out=sq, in_=x_t, func=AF.Square,
                             accum_out=ssum)
        rs_t = sm.tile([P, 1], F32, tag="rs", bufs=2, name=f"rs{st}")
        nc.scalar.activation(out=rs_t, in_=ssum, func=AF.Sqrt,
                             bias=eps_sb, scale=1.0 / DM)
        nc.vector.reciprocal(out=rs_t, in_=rs_t)
        nc.vector.tensor_scalar_mul(out=x_t, in0=x_t, scalar1=rs_t)

    def transpose_half(half):
        # xbar DMA transpose: out[p, dk, s] = normed[s, dk*128+p] -- runs on
        # the DMA engines, off the PE/DVE critical path
        for st in range(half * 4, half * 4 + 4):
            nc.sync.dma_start_transpose(
                out=ntile[:, :, st * P:(st + 1) * P], in_=x_ts[st])

    for st in range(4):
        norm_tile(st)
    transpose_half(0)

    # ---------------- projection / attention building blocks ----------------
    qt_tiles, kt_tiles = {}, {}
    vs = [None] * NS
    zT = [None] * NPAIR
    attn_tiles, vsc_tiles = {}, {}
    wo_sb = []

    def qk_groups(t):
        """4 closures: (qt/kt) x (q chunk) projection groups for pair t."""
        groups = []
        for which, ft in (("qt", t), ("kt", NPAIR + t)):
            w_t = wqk_pre.pop(ft)
            dst = qkp.tile([P, S], BF, tag=which, bufs=3, name=f"{which}{t}")
            (qt_tiles if which == "qt" else kt_tiles)[t] = dst

            def g(qc, w_t=w_t, dst=dst):
                mm_ps = ps.tile([P, QCH], F32, tag="mm", bufs=2)
                for dk in range(ND):
                    nc.tensor.matmul(
                        mm_ps, w_t[:, dk, :],
                        normT[dk][:, qc * QCH:(qc + 1) * QCH],
                        start=(dk == 0), stop=(dk == ND - 1))
                nc.vector.tensor_copy(
                    out=dst[:, qc * QCH:(qc + 1) * QCH], in_=mm_ps)

            groups += [lambda qc=qc, g=g: g(qc) for qc in range(NQC)]
        return groups

    def v_group(st, fvc):
        if vs[st] is None:
            vs[st] = xv.tile([P, DM], BF, tag=f"b{st}", name=f"v{st}")
        mm_ps = ps.tile([P, QCH], F32, tag="mm", bufs=2)
        for dk in range(ND):
            nc.tensor.matmul(
                mm_ps, normT[dk][:, st * P:(st + 1) * P],
                wv_sb[dk][:, fvc * QCH:(fvc + 1) * QCH],
                start=(dk == 0), stop=(dk == ND - 1))
        nc.vector.tensor_copy(
            out=vs[st][:, fvc * QCH:(fvc + 1) * QCH], in_=mm_ps)

    def emit_scores_ki(t, ki):
        """scores + mask + exp + rowsum + scaled-V for (pair t, k-tile ki)."""
        qt, kt = qt_tiles[t], kt_tiles[t]
        width = S - ki * P
        rsp = sm.tile([P, 2], F32, tag="rsp", bufs=4, name=f"rsp{t}_{ki}")
        if ki < NS // 2 or not MERGE_HI:
            for hl, prange in ((0, slice(0, DH)), (1, slice(DH, P))):
                sc_ps = ps.tile([P, S], F32, tag="sc", bufs=2,
                                name=f"sc{t}_{ki}_{hl}")
                kslice = kt[prange, ki * P:(ki + 1) * P]
                if ki < NS // 2:
                    # diag in q-chunk 0: [ki*P, QCH), then chunk 1
                    nc.tensor.matmul(sc_ps[:, ki * P:QCH], kslice,
                                     qt[prange, ki * P:QCH],
                                     start=True, stop=True)
                    nc.tensor.matmul(sc_ps[:, QCH:S], kslice,
                                     qt[prange, QCH:S], start=True, stop=True)
                else:
                    nc.tensor.matmul(sc_ps[:, ki * P:S], kslice,
                                     qt[prange, ki * P:S],
                                     start=True, stop=True)
                # additive causal mask on the diagonal block (DVE)
                nc.vector.tensor_add(
                    out=sc_ps[:, ki * P:(ki + 1) * P],
                    in0=sc_ps[:, ki * P:(ki + 1) * P], in1=triu_sb)
                a_t = attnp.tile([P, width], BF, tag=f"at{ki}", bufs=4,
                                 name=f"attn{t}_{hl}_{ki}")
                attn_tiles[(t, hl, ki)] = (a_t, None)
                nc.scalar.activation(out=a_t, in_=sc_ps[:, ki * P:S],
                                     func=AF.Exp, accum_out=rsp[:, hl:hl + 1])
        else:
            # both heads share one 2-bank PSUM tile (one bank each); a
            # single wide exp covers them and the DVE computes both
            # rowsums -- fewer ACT instructions and no accumulator reads
            lo = (ki - NS // 2) * P
            sc_ps = ps.tile([P, S], F32, tag="sc", bufs=2,
                            name=f"sc{t}_{ki}")
            sc3 = sc_ps.rearrange("p (h w) -> p h w", h=2)
            for hl, prange in ((0, slice(0, DH)), (1, slice(DH, P))):
                nc.tensor.matmul(
                    sc_ps[:, hl * QCH + lo:(hl + 1) * QCH],
                    kt[prange, ki * P:(ki + 1) * P],
                    qt[prange, ki * P:S], start=True, stop=True)
            tri_b = bass.AP(tensor=triu_sb.tensor, offset=triu_sb.offset,
                            ap=[list(triu_sb.ap[0]), [0, 2],
                                list(triu_sb.ap[1])])
            nc.vector.tensor_tensor(
                out=sc3[:, :, lo:lo + P], in0=sc3[:, :, lo:lo + P],
                in1=tri_b, op=ALU.add)
            a_t = attnp.tile([P, 2, width], BF, tag=f"at{ki}", bufs=4,
                             name=f"attn{t}_{ki}")
            attn_tiles[(t, 0, ki)] = (a_t, 0)
            attn_tiles[(t, 1, ki)] = (a_t, 1)
            nc.scalar.activation(out=a_t, in_=sc3[:, :, lo:QCH], func=AF.Exp)
            nc.vector.tensor_reduce(out=rsp, in_=a_t,
                                    axis=mybir.AxisListType.X, op=ALU.add)
        ri = sm.tile([P, 2], F32, tag="ri", bufs=4, name=f"ri{t}_{ki}")
        nc.vector.reciprocal(out=ri, in_=rsp)
        vsc = vscp.tile([P, P], BF, tag=f"vsc{ki}", bufs=2, name=f"vsc{t}_{ki}")
        vsc_tiles[(t, ki)] = vsc
        ri_b = bass.AP(tensor=ri.tensor, offset=ri.offset,
                       ap=[list(ri.ap[0]), list(ri.ap[1]), [0, DH]])
        nc.vector.tensor_tensor(
            out=vsc.rearrange("p (h d) -> p h d", h=2),
            in0=vs[ki][:, t * P:(t + 1) * P].rearrange("p (h d) -> p h d", h=2),
            in1=ri_b, op=ALU.mult)

    def z_group(t, qc):
        z_ps = ps.tile([P, QCH], F32, tag="z", bufs=2, name=f"z{t}_{qc}")
        kis = [ki for ki in range(NS) if ki * P < (qc + 1) * QCH]
        for hl in (0, 1):
            for i, ki in enumerate(kis):
                q0 = max(qc * QCH, ki * P)
                a_t, ahl = attn_tiles[(t, hl, ki)]
                c0, c1 = q0 - ki * P, (qc + 1) * QCH - ki * P
                rhs = a_t[:, c0:c1] if ahl is None else a_t[:, ahl, c0:c1]
                nc.tensor.matmul(
                    z_ps[hl * DH:(hl + 1) * DH, q0 - qc * QCH:QCH],
                    vsc_tiles[(t, ki)][:, hl * DH:(hl + 1) * DH],
                    rhs, start=(i == 0), stop=(i == len(kis) - 1))
        nc.vector.tensor_copy(out=zT[t][:, qc * QCH:(qc + 1) * QCH], in_=z_ps)

    # ---------------- Phase B: QK pairs 0/1, V chunk 0 + scores(0) ----------
    # q-chunk 0 of the pair-0/1 projections only needs the first transpose
    # half; emit it now so the PE isn't idle while tiles 4-7 normalize.
    g0, g1 = qk_groups(0), qk_groups(1)
    g0[0](); g0[2](); g1[0](); g1[2]()
    for st in range(4, NS):
        norm_tile(st)
    transpose_half(1)
    g0[1](); g0[3](); g1[1](); g1[3]()
    wqk_dma(2)
    wqk_dma(NPAIR + 2)
    zT[0] = ztp.tile([P, S], BF, tag="zt0", name="zT0")
    for ki in range(NS):
        v_group(ki, 0)
        emit_scores_ki(0, ki)
    # wo prefetch: w8 slots free as the last V chunk reads stream out
    def wo_dma(fk):
        w_t = w8.tile([P, DM], BF, tag=f"w{fk}", name=f"wo{fk}")
        nc.sync.dma_start(out=w_t, in_=wo[fk])
        wo_sb.append(w_t)

    # ---------------- attention iterations: pairs 1..7 ----------------
    for t in range(1, NPAIR):
        zT[t] = ztp.tile([P, S], BF, tag=f"zt{t}", name=f"zT{t}")
        fillers = [lambda t=t, qc=qc: z_group(t - 1, qc) for qc in range(NQC)]
        if t < NPAIR - 1:
            fillers += qk_groups(t + 1)
        if t < 3:
            fillers += [lambda st=st: v_group(st, 1)
                        for st in range((t - 1) * 4, t * 4)]
        if t == 1:
            fillers += [lambda fk=fk: wo_dma(fk) for fk in range(ND)]
        if t + 2 < NPAIR:
            fillers += [lambda ft=ft: wqk_dma(ft)
                        for ft in (t + 2, NPAIR + t + 2)]
        per_slot = (len(fillers) + NS - 1) // NS
        for ki in range(NS):
            emit_scores_ki(t, ki)
            for g in fillers[ki * per_slot:(ki + 1) * per_slot]:
                g()
            if t == NPAIR - 1 and ki == 5:
                # pair 7's first z chunk only needs k-tiles 0-3; emit it
                # here so the tail starts with its q0-half already done
                z_group(NPAIR - 1, 0)

    # ---------------- tail: z(7) chunk 1 + output projection ----------
    z_group(NPAIR - 1, 1)
    for st in range(NS):
        o_t = scr.tile([P, DM], BF, tag="osb", name=f"osb{st}")
        for dmc in range(NQC):
            mm_ps = ps.tile([P, QCH], F32, tag="mm", bufs=2,
                            name=f"ops{st}_{dmc}")
            for fk in range(ND):
                nc.tensor.matmul(
                    mm_ps, zT[fk][:, st * P:(st + 1) * P],
                    wo_sb[fk][:, dmc * QCH:(dmc + 1) * QCH],
                    start=(fk == 0), stop=(fk == ND - 1))
            # alternate drain engines so the final copies pipeline
            if dmc == 0:
                nc.vector.tensor_copy(
                    out=o_t[:, dmc * QCH:(dmc + 1) * QCH], in_=mm_ps)
            else:
                nc.scalar.activation(
                    out=o_t[:, dmc * QCH:(dmc + 1) * QCH], in_=mm_ps,
                    func=AF.Copy)
        nc.sync.dma_start(out=out[st * P:(st + 1) * P, :], in_=o_t)


def prep_inputs(W_qkv, W_o):
    """Host-side weight de-interleave (h, dh, 3) -> Q|K tiles, V, O; bf16."""
    W = np.asarray(W_qkv, np.float32).reshape(H, DH, 3, DM)
    Wq = W[:, :, 0, :].reshape(H * DH, DM)
    Wk = W[:, :, 1, :].reshape(H * DH, DM)
    Wv = W[:, :, 2, :].reshape(H * DH, DM)
    WqkT = np.ascontiguousarray(np.concatenate([Wq, Wk], 0).T)   # [DM, 2048]
    wqk_host = np.ascontiguousarray(
        WqkT.reshape(ND, P, 2 * NPAIR, P).transpose(2, 1, 0, 3)).astype(BF_NP)
    wv_host = np.ascontiguousarray(Wv.T).reshape(ND, P, DM).astype(BF_NP)
    wo_host = np.ascontiguousarray(
        np.asarray(W_o, np.float32).T).reshape(ND, P, DM).astype(BF_NP)
    ident = np.eye(P, dtype=np.float32).astype(BF_NP)
    idx = np.arange(P)
    # [k, q]: mask q < k (softmax over q; causal means key k sees queries >= k)
    triu_host = np.where(idx[None, :] < idx[:, None], NEG, 0.0)
    triu_host = np.ascontiguousarray(triu_host.astype(np.float32))
    return wqk_host, wv_host, wo_host, ident, triu_host


def _numpy_fallback(x, W_qkv, b_qkv, W_o, b_o):
    """Plain fp32 numpy path (only used if b_qkv is nonzero, which the
    problem's setup_inputs never produces)."""
    x = np.asarray(x, np.float32)
    normed = x * (1.0 / np.sqrt((x * x).mean(-1, keepdims=True) + EPS))
    qkv = normed @ np.asarray(W_qkv, np.float32).T + np.asarray(b_qkv, np.float32)
    qkv = qkv.reshape(*qkv.shape[:-1], H, DH, 3)
    q, k, v = qkv[..., 0], qkv[..., 1], qkv[..., 2]
    s = np.einsum('bqhd,bkhd->bhqk', q, k)
    mask = np.triu(np.ones((S, S), dtype=bool), k=1)
    s = np.where(mask[None, None], -np.inf, s)
    e = np.exp(s - s.max(axis=-2, keepdims=True))
    attn = e / e.sum(axis=-2, keepdims=True)
    z = np.einsum('bhqk,bkhd->bqhd', attn, v).reshape(*x.shape[:-1], H * DH)
    return z @ np.asarray(W_o, np.float32).T + np.asarray(b_o, np.float32)


_prog_cache = {}


def kernel(x, W_qkv, b_qkv, W_o, b_o, trace=False):
    if np.any(np.asarray(b_qkv)):
        return _numpy_fallback(x, W_qkv, b_qkv, W_o, b_o)

    wqk_host, wv_host, wo_host, ident, triu_host = prep_inputs(W_qkv, W_o)
    x_bf = np.ascontiguousarray(np.asarray(x, np.float32)).astype(BF_NP)
    if "prog" not in _prog_cache:
        _prog_cache["prog"] = build_program()
    nc = _prog_cache["prog"]

    in_maps = [{
        "xb": x_bf[bi], "wqk": wqk_host, "wv": wv_host, "wo": wo_host,
        "ident": ident, "triu_neg": triu_host,
    } for bi in range(B)]

    res = run_bass_kernel_spmd(nc, in_maps, core_ids=list(range(B)), trace=trace)
    out = np.stack([res.results[bi]["out"] for bi in range(B)]).astype(np.float32)
    out += np.asarray(b_o, np.float32)[None, None, :]
    if trace:
        kernel.last_results = res
    return out


# revision 29
# speedup vs baseline: 1.0147x; 1.0147x over previous
"""Trainium2 Bass kernel for nn_AttentionBlock (RMSNorm + QKV + causal
attention with softmax over the QUERY axis + output projection).

Sharding: data-parallel over batch. B=8 -> one batch element per NeuronCore,
no collectives.  Measured ~234 us HW exec (vs 415 us f32r baseline),
rel err 8.7e-3 vs the 2e-2 gate.

Design notes:
  * All matmul operands are bf16.  On this PE, bf16 and f32r both stream
    ~1 col/cycle (N=512 MM->MM issue gap measured 257 ns), so the bf16 win
    is NOT matmul throughput: it halves DMA/SBUF/DVE traffic, halves
    LDWEIGHTS (FWL), and -- crucially -- drops PE power enough that the
    HAM duty-cycle throttle (K=4/8 oscillation that cost the f32r baseline
    ~100 us) never engages.  fp32 is kept for all per-row scalars (1/rms,
    1/rowsum): rounding those to bf16 is a per-row systematic error that
    exp() amplifies (measured 1.5e-2 vs 8.7e-3 end-to-end).
  * Softmax work is lagged one head-pair behind the PE: during pair t the
    PE computes scores(t), the attn@V matmuls of pair t-1 (whose exp/vsc
    finished last iteration), and the QK projection of pair t+1, plus
    leftover V-chunk work -- so the PE never waits on the Scalar engine's
    exp chain.
  * Scores live transposed (sT[k, q], k on partitions) so the per-key
    softmax-over-q is a free-axis row sum: ACT exp reads the [128, width]
    PSUM row ([P, S] 2-bank tiles) and accum_out yields the row sums.
    For k-tiles >= 4 (width <= 512) both heads of the pair share one
    2-bank tile and a single wide exp covers them ([P, 2, width] strided
    AP); their row sums come from one DVE tensor_reduce.  Normalization
    is folded into V (vsc = V * 1/rowsum), never touching the attn tiles.
  * Causal mask: DVE adds a precomputed -1e30 upper-triangle onto the
    diagonal 128x128 block in PSUM (one 2-head strided add for merged
    tiles).  NOTE: a PE-side mask matmul (triu as weights, identity
    moving) also works but wastes PE issue slots; N=128 matmuls cost
    ~129 ns regardless (LDWEIGHTS-bound).
  * RMSNorm transposes write 4 x [128,128] per PSUM bank so one DVE copy
    drains four; QK projection q-chunk 0 is emitted right after the first
    transpose half so the PE isn't idle while tiles 4-7 normalize.
  * Output is written bf16 and upcast on the host (harness compares fp32).

Pitfalls hit on real HW (keep in mind when editing):
  * nc.vector.tensor_tensor_reduce crashed the device (NRT INTERNAL) --
    avoid; use ACT Square+accum_out for sum-of-squares.
  * Issuing the x DMAs from the GpSimd DGE queue added ~6 us of queue
    drain to the epilogue for no startup gain.
  * Moving the output projection's first 6 pair-contractions into the
    attention iterations (PSUM->SBUF partials + final DVE add) was a net
    LOSS: the last iterations have little true PE idle, and the tail adds
    serialize on DVE.
"""

import numpy as np
import ml_dtypes
from contextlib import ExitStack

import concourse.bacc as bacc
import concourse.bass as bass
import concourse.tile as tile
from concourse import mybir
from concourse.bass_utils import run_bass_kernel_spmd

B, S, DM, H, DH = 8, 1024, 1024, 16, 64
P = 128
EPS = 1.1920929e-07
NEG = -1e30
F32 = mybir.dt.float32
BF = mybir.dt.bfloat16
NS = S // P      # 8 s-tiles (also k-tiles)
ND = DM // P     # 8 d-chunks
NPAIR = H // 2   # 8 head pairs
QCH = 512        # moving-dim chunk (one PSUM bank of fp32)
NQC = S // QCH   # 2 q chunks

# merge both heads of a pair into one wide exp for k-tiles >= 4
MERGE_HI = True

BF_NP = ml_dtypes.bfloat16


def build_program():
    nc = bacc.Bacc("TRN2", target_bir_lowering=False, debug=False)

    xb = nc.dram_tensor("xb", [S, DM], BF, kind="ExternalInput").ap()
    # (ft, dd, dk, f): ft 0-7 = Q pair tiles, 8-15 = K pair tiles
    wqk = nc.dram_tensor("wqk", [2 * NPAIR, P, ND, P], BF, kind="ExternalInput").ap()
    wv = nc.dram_tensor("wv", [ND, P, DM], BF, kind="ExternalInput").ap()
    wo = nc.dram_tensor("wo", [ND, P, DM], BF, kind="ExternalInput").ap()
    ident = nc.dram_tensor("ident", [P, P], BF, kind="ExternalInput").ap()
    triu_neg = nc.dram_tensor("triu_neg", [P, P], F32, kind="ExternalInput").ap()
    out = nc.dram_tensor("out", [S, DM], BF, kind="ExternalOutput").ap()

    with tile.TileContext(nc) as tc:
        with ExitStack() as ctx:
            _build_body(ctx, tc, xb, wqk, wv, wo, ident, triu_neg, out)
    nc.compile()
    return nc


def _build_body(ctx, tc, xb, wqk, wv, wo, ident, triu_neg, out):
    nc = tc.nc
    AF = mybir.ActivationFunctionType
    ALU = mybir.AluOpType

    singles = ctx.enter_context(tc.tile_pool(name="singles", bufs=1))
    xv = ctx.enter_context(tc.tile_pool(name="xv", bufs=1))      # x then V
    sqp = ctx.enter_context(tc.tile_pool(name="sqp", bufs=2))    # square scratch
    nt = ctx.enter_context(tc.tile_pool(name="nt", bufs=1))      # normT
    w8 = ctx.enter_context(tc.tile_pool(name="w8", bufs=1))      # wv then wo
    wqks = ctx.enter_context(tc.tile_pool(name="wqks", bufs=4))  # wqk stream
    qkp = ctx.enter_context(tc.tile_pool(name="qkp", bufs=3))    # qt/kt tiles
    attnp = ctx.enter_context(tc.tile_pool(name="attnp", bufs=4))
    vscp = ctx.enter_context(tc.tile_pool(name="vscp", bufs=2))
    ztp = ctx.enter_context(tc.tile_pool(name="ztp", bufs=1))
    scr = ctx.enter_context(tc.tile_pool(name="scr", bufs=2))    # out tiles
    sm = ctx.enter_context(tc.tile_pool(name="sm", bufs=4))      # small stats
    ps = ctx.enter_context(tc.tile_pool(name="ps", bufs=1, space="PSUM"))

    # ---------------- Phase A: DMA + RMSNorm + transpose ----------------
    # descriptor generation costs ~0.65us per DMA on the Sync queue, so
    # issue order matters: the first 4 x tiles (gate the norm chain), then
    # ident + the pair-0/1 QK weights (needed ~15us in), then the rest.
    x_ts = [xv.tile([P, DM], BF, tag=f"b{st}", name=f"x{st}")
            for st in range(NS)]

    def x_dma(st):
        nc.sync.dma_start(out=x_ts[st], in_=xb[st * P:(st + 1) * P, :])

    for st in range(4):
        x_dma(st)
    ident_sb = singles.tile([P, P], BF, tag="ident")
    nc.sync.dma_start(out=ident_sb, in_=ident)
    triu_sb = singles.tile([P, P], F32, tag="triu")
    eps_sb = singles.tile([P, 1], F32, tag="eps")
    nc.vector.memset(eps_sb, EPS)
    # preload all three ACT function tables while the x DMA is in flight so
    # no table load lands in the norm/exp critical chain
    warm = sm.tile([P, 1], F32, tag="warm", bufs=2, name="warm")
    nc.scalar.activation(out=warm, in_=eps_sb, func=AF.Square)
    nc.scalar.activation(out=warm, in_=eps_sb, func=AF.Sqrt)
    nc.scalar.activation(out=warm, in_=eps_sb, func=AF.Exp)
    # weight prefetch behind x on the DMA queue: QK pairs 0/1, then Wv
    wqk_pre = {}

    def wqk_dma(ft):
        w_t = wqks.tile([P, ND, P], BF, tag="wqk", name=f"wqk{ft}")
        nc.sync.dma_start(out=w_t, in_=wqk[ft])
        wqk_pre[ft] = w_t

    wqk_dma(0)
    wqk_dma(NPAIR)
    x_dma(4)
    x_dma(5)
    wqk_dma(1)
    wqk_dma(NPAIR + 1)
    x_dma(6)
    x_dma(7)
    wv_sb = []
    for dk in range(ND):
        w_t = w8.tile([P, DM], BF, tag=f"w{dk}", name=f"wv{dk}")
        nc.sync.dma_start(out=w_t, in_=wv[dk])
        wv_sb.append(w_t)
    nc.sync.dma_start(out=triu_sb, in_=triu_neg)

    normT = [nt.tile([P, S], BF, tag=f"nt{dk}", name=f"normT{dk}")
             for dk in range(ND)]

    def norm_tile(st):
        x_t = x_ts[st]
        ssum = sm.tile([P, 1], F32, tag="ssum", bufs=2, name=f"ssum{st}")
        sq = sqp.tile([P, DM], BF, tag="sq", name=f"sq{st}")
        nc.scalar.activation(out=sq, in_=x_t, func=AF.Square,
                             accum_out=ssum)
        rs_t = sm.tile([P, 1], F32, tag="rs", bufs=2, name=f"rs{st}")
        nc.scalar.activation(out=rs_t, in_=ssum, func=AF.Sqrt,
                             bias=eps_sb, scale=1.0 / DM)
        nc.vector.reciprocal(out=rs_t, in_=rs_t)
        nc.vector.tensor_scalar_mul(out=x_t, in0=x_t, scalar1=rs_t)

    def transpose_half(half):
        sts = range(half * 4, half * 4 + 4)
        for dk in range(ND):
            tp = ps.tile([P, QCH], F32, tag="mm", bufs=2, name=f"tp{half}_{dk}")
            for j, st in enumerate(sts):
                nc.tensor.matmul(
                    tp[:, j * P:(j + 1) * P],
                    x_ts[st][:, dk * P:(dk + 1) * P], ident_sb,
                    start=True, stop=True)
            nc.vector.tensor_copy(
                out=normT[dk][:, half * QCH:(half + 1) * QCH], in_=tp)

    for st in range(4):
        norm_tile(st)
    transpose_half(0)

    # ---------------- projection / attention building blocks ----------------
    qt_tiles, kt_tiles = {}, {}
    vs = [None] * NS
    zT = [None] * NPAIR
    attn_tiles, vsc_tiles = {}, {}
    wo_sb = []

    def qk_groups(t):
        """4 closures: (qt/kt) x (q chunk) projection groups for pair t."""
        groups = []
        for which, ft in (("qt", t), ("kt", NPAIR + t)):
            w_t = wqk_pre.pop(ft)
            dst = qkp.tile([P, S], BF, tag=which, bufs=3, name=f"{which}{t}")
            (qt_tiles if which == "qt" else kt_tiles)[t] = dst

            def g(qc, w_t=w_t, dst=dst):
                mm_ps = ps.tile([P, QCH], F32, tag="mm", bufs=2)
                for dk in range(ND):
                    nc.tensor.matmul(
                        mm_ps, w_t[:, dk, :],
                        normT[dk][:, qc * QCH:(qc + 1) * QCH],
                        start=(dk == 0), stop=(dk == ND - 1))
                nc.vector.tensor_copy(
                    out=dst[:, qc * QCH:(qc + 1) * QCH], in_=mm_ps)

            groups += [lambda qc=qc, g=g: g(qc) for qc in range(NQC)]
        return groups

    def v_group(st, fvc):
        if vs[st] is None:
            vs[st] = xv.tile([P, DM], BF, tag=f"b{st}", name=f"v{st}")
        mm_ps = ps.tile([P, QCH], F32, tag="mm", bufs=2)
        for dk in range(ND):
            nc.tensor.matmul(
                mm_ps, normT[dk][:, st * P:(st + 1) * P],
                wv_sb[dk][:, fvc * QCH:(fvc + 1) * QCH],
                start=(dk == 0), stop=(dk == ND - 1))
        nc.vector.tensor_copy(
            out=vs[st][:, fvc * QCH:(fvc + 1) * QCH], in_=mm_ps)

    def emit_scores_ki(t, ki):
        """scores + mask + exp + rowsum + scaled-V for (pair t, k-tile ki)."""
        qt, kt = qt_tiles[t], kt_tiles[t]
        width = S - ki * P
        rsp = sm.tile([P, 2], F32, tag="rsp", bufs=4, name=f"rsp{t}_{ki}")
        if ki < NS // 2 or not MERGE_HI:
            for hl, prange in ((0, slice(0, DH)), (1, slice(DH, P))):
                sc_ps = ps.tile([P, S], F32, tag="sc", bufs=2,
                                name=f"sc{t}_{ki}_{hl}")
                kslice = kt[prange, ki * P:(ki + 1) * P]
                if ki < NS // 2:
                    # diag in q-chunk 0: [ki*P, QCH), then chunk 1
                    nc.tensor.matmul(sc_ps[:, ki * P:QCH], kslice,
                                     qt[prange, ki * P:QCH],
                                     start=True, stop=True)
                    nc.tensor.matmul(sc_ps[:, QCH:S], kslice,
                                     qt[prange, QCH:S], start=True, stop=True)
                else:
                    nc.tensor.matmul(sc_ps[:, ki * P:S], kslice,
                                     qt[prange, ki * P:S],
                                     start=True, stop=True)
                # additive causal mask on the diagonal block (DVE)
                nc.vector.tensor_add(
                    out=sc_ps[:, ki * P:(ki + 1) * P],
                    in0=sc_ps[:, ki * P:(ki + 1) * P], in1=triu_sb)
                a_t = attnp.tile([P, width], BF, tag=f"at{ki}", bufs=4,
                                 name=f"attn{t}_{hl}_{ki}")
                attn_tiles[(t, hl, ki)] = (a_t, None)
                nc.scalar.activation(out=a_t, in_=sc_ps[:, ki * P:S],
                                     func=AF.Exp, accum_out=rsp[:, hl:hl + 1])
        else:
            # both heads share one 2-bank PSUM tile (one bank each); a
            # single wide exp covers them and the DVE computes both
            # rowsums -- fewer ACT instructions and no accumulator reads
            lo = (ki - NS // 2) * P
            sc_ps = ps.tile([P, S], F32, tag="sc", bufs=2,
                            name=f"sc{t}_{ki}")
            sc3 = sc_ps.rearrange("p (h w) -> p h w", h=2)
            for hl, prange in ((0, slice(0, DH)), (1, slice(DH, P))):
                nc.tensor.matmul(
                    sc_ps[:, hl * QCH + lo:(hl + 1) * QCH],
                    kt[prange, ki * P:(ki + 1) * P],
                    qt[prange, ki * P:S], start=True, stop=True)
            tri_b = bass.AP(tensor=triu_sb.tensor, offset=triu_sb.offset,
                            ap=[list(triu_sb.ap[0]), [0, 2],
                                list(triu_sb.ap[1])])
            nc.vector.tensor_tensor(
                out=sc3[:, :, lo:lo + P], in0=sc3[:, :, lo:lo + P],
                in1=tri_b, op=ALU.add)
            a_t = attnp.tile([P, 2, width], BF, tag=f"at{ki}", bufs=4,
                             name=f"attn{t}_{ki}")
            attn_tiles[(t, 0, ki)] = (a_t, 0)
            attn_tiles[(t, 1, ki)] = (a_t, 1)
            nc.scalar.activation(out=a_t, in_=sc3[:, :, lo:QCH], func=AF.Exp)
            nc.vector.tensor_reduce(out=rsp, in_=a_t,
                                    axis=mybir.AxisListType.X, op=ALU.add)
        ri = sm.tile([P, 2], F32, tag="ri", bufs=4, name=f"ri{t}_{ki}")
        nc.vector.reciprocal(out=ri, in_=rsp)
        vsc = vscp.tile([P, P], BF, tag=f"vsc{ki}", bufs=2, name=f"vsc{t}_{ki}")
        vsc_tiles[(t, ki)] = vsc
        ri_b = bass.AP(tensor=ri.tensor, offset=ri.offset,
                       ap=[list(ri.ap[0]), list(ri.ap[1]), [0, DH]])
        nc.vector.tensor_tensor(
            out=vsc.rearrange("p (h d) -> p h d", h=2),
            in0=vs[ki][:, t * P:(t + 1) * P].rearrange("p (h d) -> p h d", h=2),
            in1=ri_b, op=ALU.mult)

    def z_group(t, qc):
        z_ps = ps.tile([P, QCH], F32, tag="z", bufs=2, name=f"z{t}_{qc}")
        kis = [ki for ki in range(NS) if ki * P < (qc + 1) * QCH]
        for hl in (0, 1):
            for i, ki in enumerate(kis):
                q0 = max(qc * QCH, ki * P)
                a_t, ahl = attn_tiles[(t, hl, ki)]
                c0, c1 = q0 - ki * P, (qc + 1) * QCH - ki * P
                rhs = a_t[:, c0:c1] if ahl is None else a_t[:, ahl, c0:c1]
                nc.tensor.matmul(
                    z_ps[hl * DH:(hl + 1) * DH, q0 - qc * QCH:QCH],
                    vsc_tiles[(t, ki)][:, hl * DH:(hl + 1) * DH],
                    rhs, start=(i == 0), stop=(i == len(kis) - 1))
        nc.vector.tensor_copy(out=zT[t][:, qc * QCH:(qc + 1) * QCH], in_=z_ps)

    # ---------------- Phase B: QK pairs 0/1, V chunk 0 + scores(0) ----------
    # q-chunk 0 of the pair-0/1 projections only needs the first transpose
    # half; emit it now so the PE isn't idle while tiles 4-7 normalize.
    g0, g1 = qk_groups(0), qk_groups(1)
    g0[0](); g0[2](); g1[0](); g1[2]()
    for st in range(4, NS):
        norm_tile(st)
    transpose_half(1)
    g0[1](); g0[3](); g1[1](); g1[3]()
    wqk_dma(2)
    wqk_dma(NPAIR + 2)
    zT[0] = ztp.tile([P, S], BF, tag="zt0", name="zT0")
    for ki in range(NS):
        v_group(ki, 0)
        emit_scores_ki(0, ki)
    # wo prefetch: w8 slots free as the last V chunk reads stream out
    def wo_dma(fk):
        w_t = w8.tile([P, DM], BF, tag=f"w{fk}", name=f"wo{fk}")
        nc.sync.dma_start(out=w_t, in_=wo[fk])
        wo_sb.append(w_t)

    # ---------------- attention iterations: pairs 1..7 ----------------
    for t in range(1, NPAIR):
        zT[t] = ztp.tile([P, S], BF, tag=f"zt{t}", name=f"zT{t}")
        fillers = [lambda t=t, qc=qc: z_group(t - 1, qc) for qc in range(NQC)]
        if t < NPAIR - 1:
            fillers += qk_groups(t + 1)
        if t < 3:
            fillers += [lambda st=st: v_group(st, 1)
                        for st in range((t - 1) * 4, t * 4)]
        if t == 1:
            fillers += [lambda fk=fk: wo_dma(fk) for fk in range(ND)]
        if t + 2 < NPAIR:
            fillers += [lambda ft=ft: wqk_dma(ft)
                        for ft in (t + 2, NPAIR + t + 2)]
        per_slot = (len(fillers) + NS - 1) // NS
        for ki in range(NS):
            emit_scores_ki(t, ki)
            for g in fillers[ki * per_slot:(ki + 1) * per_slot]:
                g()
            if t == NPAIR - 1 and ki == 5:
                # pair 7's first z chunk only needs k-tiles 0-3; emit it
                # here so the tail starts with its q0-half already done
                z_group(NPAIR - 1, 0)

    # ---------------- tail: z(7) chunk 1 + output projection ----------
    z_group(NPAIR - 1, 1)
    for st in range(NS):
        o_t = scr.tile([P, DM], BF, tag="osb", name=f"osb{st}")
        for dmc in range(NQC):
            mm_ps = ps.tile([P, QCH], F32, tag="mm", bufs=2,
                            name=f"ops{st}_{dmc}")
            for fk in range(ND):
                nc.tensor.matmul(
                    mm_ps, zT[fk][:, st * P:(st + 1) * P],
                    wo_sb[fk][:, dmc * QCH:(dmc + 1) * QCH],
                    start=(fk == 0), stop=(fk == ND - 1))
            # alternate drain engines so the final copies pipeline
            if dmc == 0:
                nc.vector.tensor_copy(
                    out=o_t[:, dmc * QCH:(dmc + 1) * QCH], in_=mm_ps)
            else:
                nc.scalar.activation(
                    out=o_t[:, dmc * QCH:(dmc + 1) * QCH], in_=mm_ps,
                    func=AF.Copy)
        nc.sync.dma_start(out=out[st * P:(st + 1) * P, :], in_=o_t)


def prep_inputs(W_qkv, W_o):
    """Host-side weight de-interleave (h, dh, 3) -> Q|K tiles, V, O; bf16."""
    W = np.asarray(W_qkv, np.float32).reshape(H, DH, 3, DM)
    Wq = W[:, :, 0, :].reshape(H * DH, DM)
    Wk = W[:, :, 1, :].reshape(H * DH, DM)
    Wv = W[:, :, 2, :].reshape(H * DH, DM)
    WqkT = np.ascontiguousarray(np.concatenate([Wq, Wk], 0).T)   # [DM, 2048]
    wqk_host = np.ascontiguousarray(
        WqkT.reshape(ND, P, 2 * NPAIR, P).transpose(2, 1, 0, 3)).astype(BF_NP)
    wv_host = np.ascontiguousarray(Wv.T).reshape(ND, P, DM).astype(BF_NP)
    wo_host = np.ascontiguousarray(
        np.asarray(W_o, np.float32).T).reshape(ND, P, DM).astype(BF_NP)
    ident = np.eye(P, dtype=np.float32).astype(BF_NP)
    idx = np.arange(P)
    # [k, q]: mask q < k (softmax over q; causal means key k sees queries >= k)
    triu_host = np.where(idx[None, :] < idx[:, None], NEG, 0.0)
    triu_host = np.ascontiguousarray(triu_host.astype(np.float32))
    return wqk_host, wv_host, wo_host, ident, triu_host


def _numpy_fallback(x, W_qkv, b_qkv, W_o, b_o):
    """Plain fp32 numpy path (only used if b_qkv is nonzero, which the
    problem's setup_inputs never produces)."""
    x = np.asarray(x, np.float32)
    normed = x * (1.0 / np.sqrt((x * x).mean(-1, keepdims=True) + EPS))
    qkv = normed @ np.asarray(W_qkv, np.float32).T + np.asarray(b_qkv, np.float32)
    qkv = qkv.reshape(*qkv.shape[:-1], H, DH, 3)
    q, k, v = qkv[..., 0], qkv[..., 1], qkv[..., 2]
    s = np.einsum('bqhd,bkhd->bhqk', q, k)
    mask = np.triu(np.ones((S, S), dtype=bool), k=1)
    s = np.where(mask[None, None], -np.inf, s)
    e = np.exp(s - s.max(axis=-2, keepdims=True))
    attn = e / e.sum(axis=-2, keepdims=True)
    z = np.einsum('bhqk,bkhd->bqhd', attn, v).reshape(*x.shape[:-1], H * DH)
    return z @ np.asarray(W_o, np.float32).T + np.asarray(b_o, np.float32)


_prog_cache = {}


def kernel(x, W_qkv, b_qkv, W_o, b_o, trace=False):
    if np.any(np.asarray(b_qkv)):
        return _numpy_fallback(x, W_qkv, b_qkv, W_o, b_o)

    wqk_host, wv_host, wo_host, ident, triu_host = prep_inputs(W_qkv, W_o)
    x_bf = np.ascontiguousarray(np.asarray(x, np.float32)).astype(BF_NP)
    if "prog" not in _prog_cache:
        _prog_cache["prog"] = build_program()
    nc = _prog_cache["prog"]

    in_maps = [{
        "xb": x_bf[bi], "wqk": wqk_host, "wv": wv_host, "wo": wo_host,
        "ident": ident, "triu_neg": triu_host,
    } for bi in range(B)]

    res = run_bass_kernel_spmd(nc, in_maps, core_ids=list(range(B)), trace=trace)
    out = np.stack([res.results[bi]["out"] for bi in range(B)]).astype(np.float32)
    out += np.asarray(b_o, np.float32)[None, None, :]
    if trace:
        kernel.last_results = res
    return out
